# revision 1
# baseline (speedup 1.0000x reference)
"""AutoCorrelationLayer kernel for 8 TRN2 NeuronCores.

Math (per reference): Q/K/V projections (D=2048, H=8 heads, DH=256),
circular cross-correlation along the head dim per (b,h,l) implemented as
half-spectrum DFT matmuls (L==S==DH==256, real inputs -> conjugate-symmetric
spectrum, and the DC bin is a per-row constant that softmax ignores, so
frequencies 1..128 suffice), softmax over the correlation axis, time-delay
aggregation (per-(b,h) 256x256 matmul with V), output projection.

Distribution: pure data-parallel over batch (B=32 -> 4 batches/core, zero
collectives).  All compute in fp16 operands with fp32 PSUM accumulation
(validated ~2.8e-3 rel err vs fp32 reference).  Activations are staged
feature-major (contraction dim on partitions) via host-side transposes of the
input shards; weights are passed transposed for the same reason.
"""

import numpy as np

import concourse.bass as bass
import concourse.mybir as mybir
import concourse.tile as tile_mod
from concourse.tile import TileContext
from concourse.vector_clock import ScopedClock
from concourse.bass_utils import run_bass_kernel_spmd

F32 = mybir.dt.float32
F16 = mybir.dt.float16
AF = mybir.ActivationFunctionType
AX = mybir.AxisListType

B, L, D, H = 32, 256, 2048, 8
DH = D // H          # 256
NCORES = 8
BPC = B // NCORES    # 4 batches per core
T = BPC * L          # 1024 tokens per core
NHALF = 2
TH = T // NHALF      # 512 tokens per half
EC = D // 128        # 16 feature chunks
DC = D // 128        # 16 contraction chunks
NF = 128             # retained spectrum bins (freqs 1..128)


def _patch_tile_drain():
    """This walrus build allows at most ONE semaphore wait per instruction;
    Tile's kernel-tail drain collects one wait per live semaphore on a single
    Drain.  Split the extras onto additional drain instructions."""
    if getattr(tile_mod.TileContext, "_drain_split_patched", False):
        return

    def _drain_and_barrier(self, tick_clock, wait_clock):
        nc = self.nc
        drain_inst = nc.sync.drain()
        wait_clock.add_sem_waits(
            drain_inst.ins, ScopedClock({None: tick_clock.global_clock})
        )
        si = drain_inst.ins.sync_info
        waits = list(si.on_wait) if si is not None and si.on_wait else []
        if len(waits) > 1:
            drain_inst.ins.sync_info = mybir.SyncInfo(
                on_wait=[waits[0]], on_update=list(si.on_update or [])
            )
            for w in waits[1:]:
                extra = nc.sync.drain()
                extra.ins.sync_info = mybir.SyncInfo(on_wait=[w], on_update=[])
        nc.all_engine_barrier()
        popped = nc._tile_sem_poison_stack.pop()
        assert popped is self._sem_poison
        nc.clear_and_free_semaphores(list(self.sems.allocated().values()))
        nc.all_engine_barrier()

    tile_mod.TileContext._drain_and_barrier = _drain_and_barrier
    tile_mod.TileContext._drain_split_patched = True


def _split_multiwaits(nc):
    """Walrus in this build rejects >1 semaphore wait per instruction.  Hoist
    extra waits onto standalone EventSemaphore NOPs inserted just before the
    offending instruction on the same engine (engines execute in order)."""
    uid = [0]
    for fn in nc.m.functions:
        for bb in fn.blocks:
            il = bb.instructions
            i = 0
            while i < len(il):
                inst = il[i]
                si = inst.sync_info
                waits = list(si.on_wait) if si is not None and si.on_wait else []
                if len(waits) > 1:
                    carriers = []
                    for w in waits[:-1]:
                        uid[0] += 1
                        es = mybir.InstEventSemaphore(
                            name=f"mwsplit_{uid[0]}",
                            engine=inst.engine,
                            ins=[], outs=[],
                            sync_info=mybir.SyncInfo(on_wait=[w], on_update=[]),
                        )
                        carriers.append(es)
                    inst.sync_info = mybir.SyncInfo(
                        on_wait=[waits[-1]], on_update=list(si.on_update or [])
                    )
                    il[i:i] = carriers
                    i += len(carriers)
                i += 1


def build_kernel():
    _patch_tile_drain()
    nc = bass.Bass()

    xq = nc.declare_dram_parameter("xq", [D, T], F32, isOutput=False)  # queries^T
    xk = nc.declare_dram_parameter("xk", [D, T], F32, isOutput=False)
    xv = nc.declare_dram_parameter("xv", [D, T], F32, isOutput=False)
    wq = nc.declare_dram_parameter("wq", [D, D], F32, isOutput=False)  # Wq^T [d,e]
    wk = nc.declare_dram_parameter("wk", [D, D], F32, isOutput=False)
    wv = nc.declare_dram_parameter("wv", [D, D], F32, isOutput=False)
    wo = nc.declare_dram_parameter("wo", [D, D], F32, isOutput=False)
    bq = nc.declare_dram_parameter("bq", [D], F32, isOutput=False)
    bk = nc.declare_dram_parameter("bk", [D], F32, isOutput=False)
    bv = nc.declare_dram_parameter("bv", [D], F32, isOutput=False)
    bo = nc.declare_dram_parameter("bo", [D], F32, isOutput=False)
    tmp = nc.declare_dram_parameter("temp", [H], F32, isOutput=False)
    dfwd = nc.declare_dram_parameter("dfwd", [2, DH, NF], F32, isOutput=False)
    dinv = nc.declare_dram_parameter("dinv", [2, NF, DH], F32, isOutput=False)
    idn = nc.declare_dram_parameter("idn", [128, 128], F32, isOutput=False)
    out = nc.declare_dram_parameter("out", [T, D], F32, isOutput=True)

    def bcast_ap(param, n):
        return bass.AP(tensor=param, offset=0, ap=[[0, 128], [1, n]])

    with TileContext(nc) as tc:
        import contextlib

        with contextlib.ExitStack() as ctx:
            consts = ctx.enter_context(tc.tile_pool(name="consts", bufs=1))
            stg = ctx.enter_context(tc.tile_pool(name="stg", bufs=4))
            wstg = ctx.enter_context(tc.tile_pool(name="wstg", bufs=6))
            persist = ctx.enter_context(tc.tile_pool(name="persist", bufs=1))
            small = ctx.enter_context(tc.tile_pool(name="small", bufs=16))

            # ---- constants ----
            ident16 = consts.tile([128, 128], F16)
            s = stg.tile([128, 128], F32, tag="stg_c")
            nc.sync.dma_start(out=s, in_=idn[:])
            nc.vector.tensor_copy(ident16, s)

            # fwd DFT mats [m%128, mc, f=128]; inv mats [f, n=256]
            fmats = []
            for i in range(2):
                t16 = consts.tile([128, 2, NF], F16, name=f"dfwd{i}", tag=f"dfwd{i}")
                for c in range(2):
                    sd = stg.tile([128, NF], F32, tag="stg_c")
                    nc.sync.dma_start(out=sd, in_=dfwd[i, c * 128:(c + 1) * 128, :])
                    nc.vector.tensor_copy(t16[:, c, :], sd)
                fmats.append(t16)
            C_sb, S_sb = fmats
            imats = []
            for i in range(2):
                t16i = consts.tile([128, DH], F16, name=f"dinv{i}", tag=f"dinv{i}")
                sd2 = stg.tile([128, DH], F32, tag="stg_c")
                nc.sync.dma_start(out=sd2, in_=dinv[i, :, :])
                nc.vector.tensor_copy(t16i, sd2)
                imats.append(t16i)
            Ci_sb, Si_sb = imats

            # biases as per-partition columns [128, EC]
            bq_sb = consts.tile([128, EC], F32)
            bk_sb = consts.tile([128, EC], F32)
            bv_sb = consts.tile([128, EC], F32)
            for bsb, bpar in ((bq_sb, bq), (bk_sb, bk), (bv_sb, bv)):
                nc.sync.dma_start(out=bsb, in_=bpar[:].rearrange("(ec p) -> p ec", p=128))
            # bo broadcast across partitions [128, D] and 1/temp columns
            bo_bc = consts.tile([128, D], F32)
            nc.sync.dma_start(out=bo_bc, in_=bcast_ap(bo, D))
            temp_bc = consts.tile([128, H], F32)
            nc.sync.dma_start(out=temp_bc, in_=bcast_ap(tmp, H))
            tinv = consts.tile([128, H], F32)
            nc.vector.reciprocal(tinv, temp_bc)
            ntinv = consts.tile([128, H], F32)
            nc.vector.tensor_scalar_mul(ntinv, tinv, -1.0)

            outf16 = persist.tile([128, EC, T], F16)      # Out_f^T [e, t]

            with tc.tile_pool(name="qkpool", bufs=1) as qkpool:
                q16 = qkpool.tile([128, EC, T], F16, tag="q16")
                k16 = qkpool.tile([128, EC, T], F16, tag="k16")

                # ---------- Q/K projections over full T (weights streamed once) ----
                for (dst16, xpar, wpar, bsb) in ((q16, xq, wq, bq_sb), (k16, xk, wk, bk_sb)):
                    with tc.tile_pool(name="xqk", bufs=1) as xqk, \
                         tc.tile_pool(name="psP", bufs=8, space="PSUM") as psP:
                        x16 = xqk.tile([128, DC, T], F16, tag="x16b")
                        pss0 = [psP.tile([128, TH], F32, tag="ps_proj",
                                         name=f"psp_0_{jt}") for jt in range(8)]
                        for dc in range(DC):
                            sw = wstg.tile([128, 512], F32, tag="stg_w")
                            nc.sync.dma_start(out=sw, in_=wpar[dc * 128:(dc + 1) * 128, 0:512])
                            for tn in range(2):
                                sx = stg.tile([128, TH], F32, tag="stg_x")
                                nc.sync.dma_start(
                                    out=sx,
                                    in_=xpar[dc * 128:(dc + 1) * 128, tn * TH:(tn + 1) * TH])
                                if dc % 2 == 0:
                                    nc.vector.tensor_copy(x16[:, dc, tn * TH:(tn + 1) * TH], sx)
                                else:
                                    nc.scalar.activation(x16[:, dc, tn * TH:(tn + 1) * TH],
                                                         sx, AF.Identity)
                            w16 = wstg.tile([128, 512], F16, tag="w16")
                            nc.vector.tensor_copy(w16, sw)
                            for j in range(4):
                                for tn in range(2):
                                    nc.tensor.matmul(
                                        pss0[j * 2 + tn], w16[:, j * 128:(j + 1) * 128],
                                        x16[:, dc, tn * TH:(tn + 1) * TH],
                                        start=(dc == 0), stop=(dc == DC - 1))
                        for ecg in range(EC // 4):
                            pss = pss0 if ecg == 0 else [
                                psP.tile([128, TH], F32, tag="ps_proj",
                                         name=f"psp_{ecg}_{jt}") for jt in range(8)]
                            if ecg > 0:
                                for dc in range(DC):
                                    sw = wstg.tile([128, 512], F32, tag="stg_w")
                                    nc.sync.dma_start(
                                        out=sw,
                                        in_=wpar[dc * 128:(dc + 1) * 128, ecg * 512:(ecg + 1) * 512])
                                    w16 = wstg.tile([128, 512], F16, tag="w16")
                                    nc.vector.tensor_copy(w16, sw)
                                    for j in range(4):
                                        for tn in range(2):
                                            nc.tensor.matmul(
                                                pss[j * 2 + tn], w16[:, j * 128:(j + 1) * 128],
                                                x16[:, dc, tn * TH:(tn + 1) * TH],
                                                start=(dc == 0), stop=(dc == DC - 1))
                            for j in range(4):
                                ec = ecg * 4 + j
                                for tn in range(2):
                                    if tn == 0:
                                        nc.scalar.activation(
                                            dst16[:, ec, tn * TH:(tn + 1) * TH],
                                            pss[j * 2 + tn], AF.Identity,
                                            bias=bsb[:, ec:ec + 1])
                                    else:
                                        nc.vector.tensor_scalar_add(
                                            dst16[:, ec, tn * TH:(tn + 1) * TH],
                                            pss[j * 2 + tn], bsb[:, ec:ec + 1])

                with tc.tile_pool(name="vpool", bufs=1) as vpool:
                    v16 = vpool.tile([128, TH // 128, D], F16)  # token-major V (per half)

                    for half in range(NHALF):
                        t0 = half * TH

                        # ---------- V projection, token-major (no transposes) ------
                        with tc.tile_pool(name="xvpool", bufs=1) as xvpool, \
                             tc.tile_pool(name="psV", bufs=6, space="PSUM") as psV:
                            xv16 = xvpool.tile([128, DC, TH], F16, tag="xv16")
                            for dc in range(DC):
                                sx = stg.tile([128, TH], F32, tag="stg_x")
                                nc.sync.dma_start(
                                    out=sx, in_=xv[dc * 128:(dc + 1) * 128, t0:t0 + TH])
                                if dc % 2 == 0:
                                    nc.vector.tensor_copy(xv16[:, dc, :], sx)
                                else:
                                    nc.scalar.activation(xv16[:, dc, :], sx, AF.Identity)
                            for ecg in range(EC // 4):
                                psv = [psV.tile([128, 512], F32, tag="ps_vproj",
                                                name=f"psv_{half}_{ecg}_{tck}")
                                       for tck in range(4)]
                                for dc in range(DC):
                                    sw = wstg.tile([128, 512], F32, tag="stg_w")
                                    nc.sync.dma_start(
                                        out=sw,
                                        in_=wv[dc * 128:(dc + 1) * 128, ecg * 512:(ecg + 1) * 512])
                                    w16 = wstg.tile([128, 512], F16, tag="w16")
                                    nc.vector.tensor_copy(w16, sw)
                                    for tck in range(4):
                                        nc.tensor.matmul(
                                            psv[tck], xv16[:, dc, tck * 128:(tck + 1) * 128],
                                            w16[:],
                                            start=(dc == 0), stop=(dc == DC - 1))
                                for tck in range(4):
                                    nc.vector.tensor_copy(
                                        v16[:, tck, ecg * 512:(ecg + 1) * 512], psv[tck])

                        # ---------- per-head spectrum corr + softmax + TDA ---------
                        with tc.tile_pool(name="hpool", bufs=2) as hpool, \
                             tc.tile_pool(name="epool", bufs=6) as epool, \
                             tc.tile_pool(name="psD", bufs=2, space="PSUM") as psD, \
                             tc.tile_pool(name="psB", bufs=3, space="PSUM") as psB, \
                             tc.tile_pool(name="psT", bufs=1, space="PSUM") as psT, \
                             tc.tile_pool(name="psO", bufs=2, space="PSUM") as psO:
                            for h in range(H):
                                qr = hpool.tile([128, TH], F16, tag="qr")
                                qi = hpool.tile([128, TH], F16, tag="qi")
                                kr = hpool.tile([128, TH], F16, tag="kr")
                                ki = hpool.tile([128, TH], F16, tag="ki")
                                for dst, src16, mat in ((qr, q16, C_sb), (qi, q16, S_sb),
                                                        (kr, k16, C_sb), (ki, k16, S_sb)):
                                    ps = psD.tile([128, TH], F32, tag="ps_dft")
                                    for mc in range(2):
                                        nc.tensor.matmul(
                                            ps, mat[:, mc, :],
                                            src16[:, h * 2 + mc, t0:t0 + TH],
                                            start=(mc == 0), stop=(mc == 1))
                                    nc.vector.tensor_copy(dst, ps)
                                pr = hpool.tile([128, TH], F16, tag="pr")
                                pi = hpool.tile([128, TH], F16, tag="pi")
                                tmp16 = hpool.tile([128, TH], F16, tag="tmp16")
                                nc.vector.tensor_mul(pr, qr, kr)
                                nc.vector.tensor_mul(tmp16, qi, ki)
                                nc.vector.tensor_add(pr, pr, tmp16)
                                nc.vector.tensor_mul(pi, qi, kr)
                                nc.vector.tensor_mul(tmp16, qr, ki)
                                nc.vector.tensor_sub(pi, pi, tmp16)

                                et16 = hpool.tile([128, 2, TH], F16, tag="et16")
                                for tck in range(TH // 128):
                                    psc = psB.tile([128, DH], F32, tag="ps_corr")
                                    nc.tensor.matmul(psc, pr[:, tck * 128:(tck + 1) * 128],
                                                     Ci_sb[:], start=True, stop=False)
                                    nc.tensor.matmul(psc, pi[:, tck * 128:(tck + 1) * 128],
                                                     Si_sb[:], start=False, stop=True)
                                    mx = small.tile([128, 1], F32, tag="mx")
                                    nc.vector.reduce_max(mx, psc[:], axis=AX.X)
                                    nbias = small.tile([128, 1], F32, tag="nbias")
                                    nc.vector.tensor_scalar_mul(nbias, mx, ntinv[:, h:h + 1])
                                    e16 = epool.tile([128, DH], F16, tag="e16")
                                    ssum = small.tile([128, 1], F32, tag="ssum")
                                    nc.scalar.activation(e16, psc[:], AF.Exp,
                                                         bias=nbias[:], scale=tinv[:, h:h + 1],
                                                         accum_out=ssum[:])
                                    rinv = small.tile([128, 1], F32, tag="rinv")
                                    nc.vector.reciprocal(rinv, ssum)
                                    en16 = epool.tile([128, DH], F16, tag="en16")
                                    nc.scalar.activation(en16, e16, AF.Identity, scale=rinv[:])
                                    for sc in range(2):
                                        pst = psT.tile([128, 128], F16, tag="ps_et")
                                        nc.tensor.transpose(
                                            pst, en16[:, sc * 128:(sc + 1) * 128], ident16[:])
                                        nc.vector.tensor_copy(
                                            et16[:, sc, tck * 128:(tck + 1) * 128], pst)
                                # TDA: Outf^T[i, t] += Vp[s,i].T @ E^T[s,t] per local batch
                                for b in range(TH // L):
                                    for ic in range(2):
                                        pso = psO.tile([128, L], F32, tag="ps_tda")
                                        for sc in range(2):
                                            nc.tensor.matmul(
                                                pso,
                                                v16[:, b * 2 + sc,
                                                    h * DH + ic * 128:h * DH + (ic + 1) * 128],
                                                et16[:, sc, b * L:(b + 1) * L],
                                                start=(sc == 0), stop=(sc == 1))
                                        nc.scalar.activation(
                                            outf16[:, h * 2 + ic, t0 + b * L:t0 + (b + 1) * L],
                                            pso, AF.Identity,
                                            bias=bv_sb[:, h * 2 + ic:h * 2 + ic + 1])

            # ---------- output projection: Y[t,o] = Outf^T.T @ Wo^T + bo ----------
            with tc.tile_pool(name="wopool", bufs=1) as wopool, \
                 tc.tile_pool(name="ypool", bufs=4) as ypool, \
                 tc.tile_pool(name="psY", bufs=8, space="PSUM") as psY:
                wo16 = wopool.tile([128, EC, D], F16)
                for ec in range(EC):
                    sw = wopool.tile([128, D], F32, tag="stg_wo", bufs=3, name=f"stg_wo_{ec}")
                    nc.sync.dma_start(out=sw, in_=wo[ec * 128:(ec + 1) * 128, :])
                    if ec % 2 == 0:
                        nc.vector.tensor_copy(wo16[:, ec, :], sw)
                    else:
                        nc.scalar.activation(wo16[:, ec, :], sw, AF.Identity)
                for tck in range(T // 128):
                    pss = [psY.tile([128, 512], F32, tag="ps_y", name=f"ps_y_{tck}_{i}")
                           for i in range(4)]
                    for ec in range(EC):
                        for oc in range(4):
                            nc.tensor.matmul(pss[oc], outf16[:, ec, tck * 128:(tck + 1) * 128],
                                             wo16[:, ec, oc * 512:(oc + 1) * 512],
                                             start=(ec == 0), stop=(ec == EC - 1))
                    for oc in range(4):
                        yt = ypool.tile([128, 512], F32, tag="yt")
                        nc.vector.tensor_add(yt, pss[oc], bo_bc[:, oc * 512:(oc + 1) * 512])
                        nc.sync.dma_start(out=out[tck * 128:(tck + 1) * 128, oc * 512:(oc + 1) * 512],
                                          in_=yt)
    _split_multiwaits(nc)
    return nc


_NC_CACHE = None


def _get_nc():
    global _NC_CACHE
    if _NC_CACHE is None:
        _NC_CACHE = build_kernel()
    return _NC_CACHE


def _dft_consts():
    m = np.arange(DH, dtype=np.float64)
    f = np.arange(1, NF + 1, dtype=np.float64)   # freqs 1..128 (DC dropped: softmax-invariant)
    ang_f = 2.0 * np.pi * np.outer(m, f) / DH
    C = np.cos(ang_f)            # [m, NF]
    S = -np.sin(ang_f)
    n = np.arange(DH, dtype=np.float64)
    w = np.where(f < NF, 2.0, 1.0)[:, None]      # conjugate-symmetry weights; Nyquist = 1
    ang_i = 2.0 * np.pi * np.outer(f, n) / DH
    Ci = w * np.cos(ang_i) / DH  # [NF, n]
    Si = -w * np.sin(ang_i) / DH
    dfwd = np.stack([C, S]).astype(np.float32)
    dinv = np.stack([Ci, Si]).astype(np.float32)
    return dfwd, dinv


def make_in_maps(inputs):
    dfwd, dinv = _dft_consts()
    idn = np.eye(128, dtype=np.float32)
    shared = {
        "wq": np.ascontiguousarray(inputs["Wq"].T).astype(np.float32, copy=False),
        "wk": np.ascontiguousarray(inputs["Wk"].T).astype(np.float32, copy=False),
        "wv": np.ascontiguousarray(inputs["Wv"].T).astype(np.float32, copy=False),
        "wo": np.ascontiguousarray(inputs["Wo"].T).astype(np.float32, copy=False),
        "bq": np.asarray(inputs["bq"], np.float32),
        "bk": np.asarray(inputs["bk"], np.float32),
        "bv": np.asarray(inputs["bv"], np.float32),
        "bo": np.asarray(inputs["bo"], np.float32),
        "temp": np.ascontiguousarray(np.asarray(inputs["temperature"], np.float32).reshape(H)),
        "dfwd": dfwd,
        "dinv": dinv,
        "idn": idn,
    }
    in_maps = []
    for c in range(NCORES):
        sl = slice(c * BPC, (c + 1) * BPC)
        m = dict(shared)
        for key, name in (("queries", "xq"), ("keys", "xk"), ("values", "xv")):
            x = np.asarray(inputs[key], np.float32)[sl].reshape(T, D)
            m[name] = np.ascontiguousarray(x.T)
        in_maps.append(m)
    return in_maps


def kernel(**inputs):
    nc = _get_nc()
    in_maps = make_in_maps(inputs)
    res = run_bass_kernel_spmd(nc, in_maps, list(range(NCORES)))
    outs = [res.results[i]["out"].reshape(BPC, L, D) for i in range(NCORES)]
    return np.concatenate(outs, axis=0).astype(np.float32, copy=False)



# revision 16
# speedup vs baseline: 1.2071x; 1.2071x over previous
"""AutoCorrelationLayer kernel for 8 TRN2 NeuronCores.

Math (per reference): Q/K/V projections (D=2048, H=8 heads, DH=256),
circular cross-correlation along the head dim per (b,h,l), softmax over the
correlation axis, time-delay aggregation (per-(b,h) 256x256 matmul with V),
output projection.

Key restructurings vs a direct translation:
- The forward DFT acts on the projection OUTPUT features, so it is folded
  into Wq/Wk on the host (WqF = Wq^T @ blockdiag(F)); the device projections
  directly produce the half-spectrum (cos/sin) features at zero extra cost.
  The DC bin is softmax-invariant and dropped; freqs 1..128 retained.
- bv contributes bv @ Wo^T to the output (softmax rows sum to 1), so it is
  folded into bo on the host; V/TDA paths carry no bias.
- All weights and activations are converted to fp16 on the host: no on-chip
  casts, half the HBM traffic. PSUM accumulation is fp32.
- Distribution: pure data-parallel over batch (B=32 -> 4 batches/core).
- Scheduling: V-projection (second half) and O-projection (first half)
  matmuls are interleaved into the per-head correlation/softmax/TDA phases so
  the tensor engine stays continuously busy (avoids p-state down-ramp).
"""

import numpy as np

import concourse.bass as bass
import concourse.mybir as mybir
import concourse.tile as tile_mod
from concourse.tile import TileContext
from concourse.vector_clock import ScopedClock
from concourse.bass_utils import run_bass_kernel_spmd

F32 = mybir.dt.float32
F16 = mybir.dt.float16
AF = mybir.ActivationFunctionType
AX = mybir.AxisListType

B, L, D, H = 32, 256, 2048, 8
DH = D // H          # 256
NCORES = 8
BPC = B // NCORES    # 4 batches per core
T = BPC * L          # 1024 tokens per core
NHALF = 2
TH = T // NHALF      # 512 tokens per half
EC = D // 128        # 16 feature chunks
DC = D // 128        # 16 contraction chunks
NF = 128             # retained spectrum bins (freqs 1..128)


def _patch_tile_drain():
    """This walrus build allows at most ONE semaphore wait per instruction;
    Tile's kernel-tail drain collects one wait per live semaphore on a single
    Drain.  Split the extras onto additional drain instructions."""
    if getattr(tile_mod.TileContext, "_drain_split_patched", False):
        return

    def _drain_and_barrier(self, tick_clock, wait_clock):
        nc = self.nc
        drain_inst = nc.sync.drain()
        wait_clock.add_sem_waits(
            drain_inst.ins, ScopedClock({None: tick_clock.global_clock})
        )
        si = drain_inst.ins.sync_info
        waits = list(si.on_wait) if si is not None and si.on_wait else []
        if len(waits) > 1:
            drain_inst.ins.sync_info = mybir.SyncInfo(
                on_wait=[waits[0]], on_update=list(si.on_update or [])
            )
            for w in waits[1:]:
                extra = nc.sync.drain()
                extra.ins.sync_info = mybir.SyncInfo(on_wait=[w], on_update=[])
        nc.all_engine_barrier()
        popped = nc._tile_sem_poison_stack.pop()
        assert popped is self._sem_poison
        nc.clear_and_free_semaphores(list(self.sems.allocated().values()))
        nc.all_engine_barrier()

    tile_mod.TileContext._drain_and_barrier = _drain_and_barrier
    tile_mod.TileContext._drain_split_patched = True


def _split_multiwaits(nc):
    """Walrus in this build rejects >1 semaphore wait per instruction.  Hoist
    extra waits onto standalone EventSemaphore NOPs inserted just before the
    offending instruction on the same engine (engines execute in order)."""
    uid = [0]
    for fn in nc.m.functions:
        for bb in fn.blocks:
            il = bb.instructions
            i = 0
            while i < len(il):
                inst = il[i]
                si = inst.sync_info
                waits = list(si.on_wait) if si is not None and si.on_wait else []
                if len(waits) > 1:
                    carriers = []
                    for w in waits[:-1]:
                        uid[0] += 1
                        es = mybir.InstEventSemaphore(
                            name=f"mwsplit_{uid[0]}",
                            engine=inst.engine,
                            ins=[], outs=[],
                            sync_info=mybir.SyncInfo(on_wait=[w], on_update=[]),
                        )
                        carriers.append(es)
                    inst.sync_info = mybir.SyncInfo(
                        on_wait=[waits[-1]], on_update=list(si.on_update or [])
                    )
                    il[i:i] = carriers
                    i += len(carriers)
                i += 1


def _drive(gen, n):
    for _ in range(n):
        try:
            next(gen)
        except StopIteration:
            return


def build_kernel():
    _patch_tile_drain()
    nc = bass.Bass()

    xq = nc.declare_dram_parameter("xq", [D, T], F16, isOutput=False)  # queries^T
    xk = nc.declare_dram_parameter("xk", [D, T], F16, isOutput=False)
    xv = nc.declare_dram_parameter("xv", [D, T], F16, isOutput=False)
    wq = nc.declare_dram_parameter("wq", [D, D], F16, isOutput=False)  # (Wq^T@F) [d,e']
    wk = nc.declare_dram_parameter("wk", [D, D], F16, isOutput=False)
    wv = nc.declare_dram_parameter("wv", [D, D], F16, isOutput=False)  # Wv^T
    wo = nc.declare_dram_parameter("wo", [D, D], F16, isOutput=False)  # Wo^T
    bq = nc.declare_dram_parameter("bq", [D], F32, isOutput=False)     # bq@F
    bk = nc.declare_dram_parameter("bk", [D], F32, isOutput=False)
    bo = nc.declare_dram_parameter("bo", [D], F32, isOutput=False)     # bo + Wo@bv
    tmp = nc.declare_dram_parameter("temp", [H], F32, isOutput=False)
    dinv = nc.declare_dram_parameter("dinv", [2, NF, DH], F16, isOutput=False)
    idn = nc.declare_dram_parameter("idn", [128, 128], F16, isOutput=False)
    out = nc.declare_dram_parameter("out", [T, D], F32, isOutput=True)

    def bcast_ap(param, n):
        return bass.AP(tensor=param, offset=0, ap=[[0, 128], [1, n]])

    with TileContext(nc) as tc:
        import contextlib

        with contextlib.ExitStack() as ctx:
            consts = ctx.enter_context(tc.tile_pool(name="consts", bufs=1))
            qkpool = ctx.enter_context(tc.tile_pool(name="qkpool", bufs=1))
            outfpool = ctx.enter_context(tc.tile_pool(name="outfpool", bufs=1))

            # ---- constants (all fp16 direct from host; no casts) ----
            ident16 = consts.tile([128, 128], F16)
            nc.sync.dma_start(out=ident16, in_=idn[:])
            Ci_sb = consts.tile([128, DH], F16, name="dinv0")
            Si_sb = consts.tile([128, DH], F16, name="dinv1")
            nc.sync.dma_start(out=Ci_sb, in_=dinv[0, :, :])
            nc.sync.dma_start(out=Si_sb, in_=dinv[1, :, :])

            # spectrum-domain biases as per-partition columns [128, EC]
            bq_sb = consts.tile([128, EC], F32)
            bk_sb = consts.tile([128, EC], F32)
            for bsb, bpar in ((bq_sb, bq), (bk_sb, bk)):
                nc.sync.dma_start(out=bsb, in_=bpar[:].rearrange("(ec p) -> p ec", p=128))
            temp_bc = consts.tile([128, H], F32)
            nc.sync.dma_start(out=temp_bc, in_=bcast_ap(tmp, H))
            tinv = consts.tile([128, H], F32)
            nc.vector.reciprocal(tinv, temp_bc)
            ntinv = consts.tile([128, H], F32)
            nc.vector.tensor_scalar_mul(ntinv, tinv, -1.0)

            q16 = qkpool.tile([128, EC, T], F16, tag="q16")   # spectrum Q^T [e', t]
            k16 = qkpool.tile([128, EC, T], F16, tag="k16")
            outf16 = outfpool.tile([128, EC, T], F16)         # Out_f^T [e, t]

            # ---------- Phase 1: Q/K spectrum projections over full T ----------
            with tc.tile_pool(name="xqk", bufs=1) as xqk, \
                 tc.tile_pool(name="wstg1", bufs=1) as wstg1, \
                 tc.tile_pool(name="psP", bufs=8, space="PSUM") as psP:
                xq16 = xqk.tile([128, DC, T], F16, tag="xq16")
                xk16 = xqk.tile([128, DC, T], F16, tag="xk16")
                for dc in range(DC):
                    nc.sync.dma_start(out=xq16[:, dc, :],
                                      in_=xq[dc * 128:(dc + 1) * 128, :])
                for dc in range(DC):
                    nc.sync.dma_start(out=xk16[:, dc, :],
                                      in_=xk[dc * 128:(dc + 1) * 128, :])
                for (dst16, x16, wpar, bsb) in ((q16, xq16, wq, bq_sb),
                                                (k16, xk16, wk, bk_sb)):
                    for ecg in range(EC // 4):
                        pss = [psP.tile([128, TH], F32, tag="ps_proj",
                                        name=f"psp_{dst16.name}_{ecg}_{jt}")
                               for jt in range(8)]
                        for dc in range(DC):
                            w16 = wstg1.tile([128, 512], F16, tag="w16", bufs=10)
                            nc.sync.dma_start(
                                out=w16,
                                in_=wpar[dc * 128:(dc + 1) * 128,
                                         ecg * 512:(ecg + 1) * 512])
                            for j in range(4):
                                for tn in range(2):
                                    nc.tensor.matmul(
                                        pss[j * 2 + tn], w16[:, j * 128:(j + 1) * 128],
                                        x16[:, dc, tn * TH:(tn + 1) * TH],
                                        start=(dc == 0), stop=(dc == DC - 1))
                        for j in range(4):
                            ec = ecg * 4 + j
                            for tn in range(2):
                                if j < 2:
                                    nc.scalar.activation(
                                        dst16[:, ec, tn * TH:(tn + 1) * TH],
                                        pss[j * 2 + tn], AF.Identity,
                                        bias=bsb[:, ec:ec + 1])
                                else:
                                    nc.vector.tensor_scalar_add(
                                        dst16[:, ec, tn * TH:(tn + 1) * TH],
                                        pss[j * 2 + tn], bsb[:, ec:ec + 1])

            # ---------- Phases 2-5 ----------
            with tc.tile_pool(name="hpool", bufs=1) as hpool, \
                 tc.tile_pool(name="epool", bufs=1) as epool, \
                 tc.tile_pool(name="small", bufs=8) as small, \
                 tc.tile_pool(name="ypool", bufs=3) as ypool, \
                 tc.tile_pool(name="vpool", bufs=1) as vpool, \
                 tc.tile_pool(name="wstg2", bufs=1) as wstg2:
                v16 = [vpool.tile([128, TH // 128, D], F16, tag=f"v16_{hf}",
                                  name=f"v16_{hf}")
                       for hf in range(2)]

                def v_steps(half):
                    """V projection for one half, token-major; yields ~36x."""
                    t0 = half * TH
                    for ecg in range(EC // 4):
                        psv = [psV.tile([128, 512], F32, tag="ps_vproj",
                                        name=f"psv_{half}_{ecg}_{t}")
                               for t in range(4)]
                        for dc in range(DC):
                            w16 = wstg2.tile([128, 512], F16, tag="wv16", bufs=7)
                            nc.sync.dma_start(
                                out=w16,
                                in_=wv[dc * 128:(dc + 1) * 128,
                                       ecg * 512:(ecg + 1) * 512])
                            for tck in range(4):
                                nc.tensor.matmul(
                                    psv[tck],
                                    xv16[:, dc, t0 + tck * 128:t0 + (tck + 1) * 128],
                                    w16[:], start=(dc == 0), stop=(dc == DC - 1))
                            if dc % 2 == 1:
                                yield
                        for tck in range(4):
                            dst = v16[half][:, tck, ecg * 512:(ecg + 1) * 512]
                            if tck % 2 == 0:
                                nc.scalar.activation(dst, psv[tck], AF.Identity)
                            else:
                                nc.vector.tensor_copy(dst, psv[tck])
                        yield

                def o_steps(half, nbank):
                    """O projection for one half; nbank PSUM banks; yields."""
                    t0 = half * TH
                    ocw = nbank // 4          # 512-col groups per pass
                    for ocp in range(4 // ocw):
                        psy = [psY[half].tile([128, 512], F32, tag="ps_y",
                                              name=f"psy_{half}_{ocp}_{i}")
                               for i in range(nbank)]
                        c0 = ocp * ocw * 512
                        boc = []
                        for oc in range(ocw):
                            bt = ypool.tile([128, 512], F32, tag="boc", bufs=2,
                                            name=f"boc_{half}_{ocp}_{oc}")
                            nc.sync.dma_start(
                                out=bt,
                                in_=bass.AP(tensor=bo, offset=c0 + oc * 512,
                                            ap=[[0, 128], [1, 512]]))
                            boc.append(bt)
                        for ec in range(EC):
                            w16 = wstg2.tile([128, ocw * 512], F16,
                                             tag=f"wo16_{ocw}", bufs=8 // ocw)
                            for oc in range(ocw):
                                nc.sync.dma_start(
                                    out=w16[:, oc * 512:(oc + 1) * 512],
                                    in_=wo[ec * 128:(ec + 1) * 128,
                                           c0 + oc * 512:c0 + (oc + 1) * 512])
                            for tck in range(4):
                                for oc in range(ocw):
                                    nc.tensor.matmul(
                                        psy[tck * ocw + oc],
                                        outf16[:, ec, t0 + tck * 128:t0 + (tck + 1) * 128],
                                        w16[:, oc * 512:(oc + 1) * 512],
                                        start=(ec == 0), stop=(ec == EC - 1))
                            if ec % 2 == 1:
                                yield
                        nsplit = 2 * ocw
                        for tck in range(4):
                            for oc in range(ocw):
                                cc = c0 + oc * 512
                                yt = ypool.tile([128, 512], F32, tag="yt", bufs=3)
                                nc.vector.tensor_add(yt, psy[tck * ocw + oc],
                                                     boc[oc][:])
                                for j in range(nsplit):
                                    w = 512 // nsplit
                                    nc.sync.dma_start(
                                        out=out[t0 + tck * 128:t0 + (tck + 1) * 128,
                                                cc + j * w:cc + (j + 1) * w],
                                        in_=yt[:, j * w:(j + 1) * w])
                        yield

                def emit_b_head(h, half, fill):
                    """Correlation spectrum -> softmax -> TDA for one head/half.
                    fill: generator driven at stall-prone points."""
                    t0 = half * TH
                    qr = q16[:, 2 * h, t0:t0 + TH]
                    qi = q16[:, 2 * h + 1, t0:t0 + TH]
                    kr = k16[:, 2 * h, t0:t0 + TH]
                    ki = k16[:, 2 * h + 1, t0:t0 + TH]
                    pr = hpool.tile([128, TH], F16, tag="pr", bufs=2)
                    pi = hpool.tile([128, TH], F16, tag="pi", bufs=2)
                    tA = hpool.tile([128, TH], F16, tag="tA", bufs=1)
                    tB = hpool.tile([128, TH], F16, tag="tB", bufs=1)
                    nc.vector.tensor_mul(pr, qr, kr)
                    nc.vector.tensor_mul(tA, qi, ki)
                    nc.vector.tensor_mul(pi, qi, kr)
                    nc.vector.tensor_mul(tB, qr, ki)
                    nc.vector.tensor_add(pr, pr, tA)
                    nc.vector.tensor_sub(pi, pi, tB)

                    # spectrum -> corr (pairs of tck share one PSUM bank; the
                    # pair tile is reused only across heads, after its reads)
                    pscp = []
                    for hp in range(2):
                        pp = psB.tile([128, 2, DH], F32, tag="ps_corr")
                        for j in range(2):
                            tck = hp * 2 + j
                            nc.tensor.matmul(pp[:, j, :],
                                             pr[:, tck * 128:(tck + 1) * 128],
                                             Ci_sb[:], start=True, stop=False)
                            nc.tensor.matmul(pp[:, j, :],
                                             pi[:, tck * 128:(tck + 1) * 128],
                                             Si_sb[:], start=False, stop=True)
                        pscp.append(pp)
                        _drive(fill, 1)
                    _drive(fill, 1)

                    pst = psT.tile([128, 8, 128], F16, tag="ps_et")
                    et16 = epool.tile([128, 2, TH], F16, tag="et16", bufs=2)
                    for tck in range(4):
                        psc = pscp[tck // 2][:, tck % 2, :]
                        mx = small.tile([128, 1], F32, tag="mx")
                        nc.vector.reduce_max(mx, psc, axis=AX.X)
                        nbias = small.tile([128, 1], F32, tag="nbias")
                        nc.vector.tensor_scalar_mul(nbias, mx, ntinv[:, h:h + 1])
                        e16 = epool.tile([128, DH], F16, tag="e16", bufs=2)
                        ssum = small.tile([128, 1], F32, tag="ssum")
                        nc.scalar.activation(e16, psc, AF.Exp,
                                             bias=nbias[:], scale=tinv[:, h:h + 1],
                                             accum_out=ssum[:])
                        rinv = small.tile([128, 1], F32, tag="rinv")
                        nc.vector.reciprocal(rinv, ssum)
                        en16 = epool.tile([128, DH], F16, tag="en16", bufs=2)
                        nc.vector.tensor_scalar_mul(en16, e16, rinv)
                        for sc in range(2):
                            nc.tensor.transpose(pst[:, sc * 4 + tck, :],
                                                en16[:, sc * 128:(sc + 1) * 128],
                                                ident16[:])
                    for sc in range(2):
                        nc.vector.tensor_copy(
                            et16[:, sc, :].rearrange("p (tck c) -> p tck c", c=128),
                            pst[:, sc * 4:(sc + 1) * 4, :])
                    _drive(fill, 1)

                    # TDA: Outf^T[i, t] += Vp[s,i].T @ E^T[s,t] per local batch
                    for b in range(TH // L):
                        pso = psO.tile([128, 2, L], F32, tag="ps_tda")
                        for ic in range(2):
                            for sc in range(2):
                                nc.tensor.matmul(
                                    pso[:, ic, :],
                                    v16[half][:, b * 2 + sc,
                                              h * DH + ic * 128:h * DH + (ic + 1) * 128],
                                    et16[:, sc, b * L:(b + 1) * L],
                                    start=(sc == 0), stop=(sc == 1))
                        for ic in range(2):
                            nc.scalar.activation(
                                outf16[:, 2 * h + ic, t0 + b * L:t0 + (b + 1) * L],
                                pso[:, ic, :], AF.Identity)
                        _drive(fill, 1)

                with tc.tile_pool(name="psB", bufs=2, space="PSUM") as psB, \
                     tc.tile_pool(name="psT", bufs=1, space="PSUM") as psT, \
                     tc.tile_pool(name="psO", bufs=1, space="PSUM") as psO:
                    with tc.tile_pool(name="xvpool", bufs=1) as xvpool, \
                         tc.tile_pool(name="psV", bufs=4, space="PSUM") as psV:
                        xv16 = xvpool.tile([128, DC, T], F16)
                        for dc in range(DC):
                            nc.sync.dma_start(out=xv16[:, dc, :],
                                              in_=xv[dc * 128:(dc + 1) * 128, :])
                        # Phase 2: V(h0) standalone
                        for _ in v_steps(0):
                            pass
                        # Phase 3: B(h0) interleaved with V(h1)
                        vgen = v_steps(1)
                        for h in range(H):
                            emit_b_head(h, 0, vgen)
                        _drive(vgen, 40)

                    # Phase 4: B(h1) interleaved with O(h0)
                    psY = {0: None, 1: None}
                    with tc.tile_pool(name="psY0", bufs=4, space="PSUM") as psY0:
                        psY[0] = psY0
                        ogen = o_steps(0, 4)
                        for h in range(H):
                            emit_b_head(h, 1, ogen)
                        _drive(ogen, 40)

                # Phase 5: O(h1) standalone with 8 banks
                with tc.tile_pool(name="psY1", bufs=8, space="PSUM") as psY1:
                    psY[1] = psY1
                    for _ in o_steps(1, 8):
                        pass

    _split_multiwaits(nc)
    return nc


_NC_CACHE = None


def _get_nc():
    global _NC_CACHE
    if _NC_CACHE is None:
        _NC_CACHE = build_kernel()
    return _NC_CACHE


def _dft_consts():
    m = np.arange(DH, dtype=np.float64)
    f = np.arange(1, NF + 1, dtype=np.float64)   # freqs 1..128 (DC dropped)
    ang_f = 2.0 * np.pi * np.outer(m, f) / DH
    C = np.cos(ang_f)             # [m, NF]
    S = -np.sin(ang_f)
    F = np.concatenate([C, S], axis=1)           # [DH, 2*NF] per-head fold
    n = np.arange(DH, dtype=np.float64)
    w = np.where(f < NF, 2.0, 1.0)[:, None]      # conj-symmetry wts; Nyquist=1
    ang_i = 2.0 * np.pi * np.outer(f, n) / DH
    Ci = w * np.cos(ang_i) / DH   # [NF, n]
    Si = -w * np.sin(ang_i) / DH
    dinv = np.stack([Ci, Si]).astype(np.float16)
    return F, dinv


def _fold_w(Wt, F):
    """Wt [D, H*DH] -> per-head Wt_block @ F, fp16."""
    Wf = np.einsum("dhm,me->dhe", Wt.reshape(D, H, DH).astype(np.float32),
                   F.astype(np.float32))
    return np.ascontiguousarray(Wf.reshape(D, D)).astype(np.float16)


def make_in_maps(inputs):
    F, dinv = _dft_consts()
    idn = np.eye(128, dtype=np.float16)
    Wq = np.asarray(inputs["Wq"], np.float32)
    Wk = np.asarray(inputs["Wk"], np.float32)
    bqF = (np.asarray(inputs["bq"], np.float64).reshape(H, DH) @ F).reshape(D)
    bkF = (np.asarray(inputs["bk"], np.float64).reshape(H, DH) @ F).reshape(D)
    boF = np.asarray(inputs["bo"], np.float64) + \
        np.asarray(inputs["Wo"], np.float64) @ np.asarray(inputs["bv"], np.float64)
    shared = {
        "wq": _fold_w(Wq.T, F),
        "wk": _fold_w(Wk.T, F),
        "wv": np.ascontiguousarray(np.asarray(inputs["Wv"], np.float32).T).astype(np.float16),
        "wo": np.ascontiguousarray(np.asarray(inputs["Wo"], np.float32).T).astype(np.float16),
        "bq": bqF.astype(np.float32),
        "bk": bkF.astype(np.float32),
        "bo": boF.astype(np.float32),
        "temp": np.ascontiguousarray(np.asarray(inputs["temperature"], np.float32).reshape(H)),
        "dinv": dinv,
        "idn": idn,
    }
    in_maps = []
    for c in range(NCORES):
        sl = slice(c * BPC, (c + 1) * BPC)
        m = dict(shared)
        for key, name in (("queries", "xq"), ("keys", "xk"), ("values", "xv")):
            x = np.asarray(inputs[key], np.float32)[sl].reshape(T, D)
            m[name] = np.ascontiguousarray(x.T).astype(np.float16)
        in_maps.append(m)
    return in_maps


def kernel(**inputs):
    nc = _get_nc()
    in_maps = make_in_maps(inputs)
    res = run_bass_kernel_spmd(nc, in_maps, list(range(NCORES)))
    outs = [res.results[i]["out"].reshape(BPC, L, D) for i in range(NCORES)]
    return np.concatenate(outs, axis=0).astype(np.float32, copy=False)


# revision 19
# speedup vs baseline: 1.2834x; 1.0632x over previous
"""AutoCorrelationLayer kernel for 8 TRN2 NeuronCores.

Math (per reference): Q/K/V projections (D=2048, H=8 heads, DH=256),
circular cross-correlation along the head dim per (b,h,l), softmax over the
correlation axis, time-delay aggregation (per-(b,h) 256x256 matmul with V),
output projection.

Key restructurings vs a direct translation:
- The forward DFT acts on the projection OUTPUT features, so it is folded
  into Wq/Wk on the host (WqF = Wq^T @ blockdiag(F)); the device projections
  directly produce the half-spectrum (cos/sin) features at zero extra cost.
  The DC bin is softmax-invariant and dropped; freqs 1..128 retained.
- bv contributes bv @ Wo^T to the output (softmax rows sum to 1), so it is
  folded into bo on the host; V/TDA paths carry no bias.
- All weights and activations are converted to fp16 on the host: no on-chip
  casts, half the HBM traffic. PSUM accumulation is fp32.
- Distribution: pure data-parallel over batch (B=32 -> 4 batches/core).
- Scheduling: V-projection (second half) and O-projection (first half)
  matmuls are interleaved into the per-head correlation/softmax/TDA phases so
  the tensor engine stays continuously busy (avoids p-state down-ramp).
"""

import numpy as np

import concourse.bass as bass
import concourse.mybir as mybir
import concourse.tile as tile_mod
from concourse.tile import TileContext
from concourse.vector_clock import ScopedClock
from concourse.bass_utils import run_bass_kernel_spmd

F32 = mybir.dt.float32
F16 = mybir.dt.float16
AF = mybir.ActivationFunctionType
AX = mybir.AxisListType

B, L, D, H = 32, 256, 2048, 8
DH = D // H          # 256
NCORES = 8
BPC = B // NCORES    # 4 batches per core
T = BPC * L          # 1024 tokens per core
NHALF = 2
TH = T // NHALF      # 512 tokens per half
EC = D // 128        # 16 feature chunks
DC = D // 128        # 16 contraction chunks
NF = 128             # retained spectrum bins (freqs 1..128)


def _patch_tile_drain():
    """This walrus build allows at most ONE semaphore wait per instruction;
    Tile's kernel-tail drain collects one wait per live semaphore on a single
    Drain.  Split the extras onto additional drain instructions."""
    if getattr(tile_mod.TileContext, "_drain_split_patched", False):
        return

    def _drain_and_barrier(self, tick_clock, wait_clock):
        nc = self.nc
        drain_inst = nc.sync.drain()
        wait_clock.add_sem_waits(
            drain_inst.ins, ScopedClock({None: tick_clock.global_clock})
        )
        si = drain_inst.ins.sync_info
        waits = list(si.on_wait) if si is not None and si.on_wait else []
        if len(waits) > 1:
            drain_inst.ins.sync_info = mybir.SyncInfo(
                on_wait=[waits[0]], on_update=list(si.on_update or [])
            )
            for w in waits[1:]:
                extra = nc.sync.drain()
                extra.ins.sync_info = mybir.SyncInfo(on_wait=[w], on_update=[])
        nc.all_engine_barrier()
        popped = nc._tile_sem_poison_stack.pop()
        assert popped is self._sem_poison
        nc.clear_and_free_semaphores(list(self.sems.allocated().values()))
        nc.all_engine_barrier()

    tile_mod.TileContext._drain_and_barrier = _drain_and_barrier
    tile_mod.TileContext._drain_split_patched = True


def _split_multiwaits(nc):
    """Walrus in this build rejects >1 semaphore wait per instruction.  Hoist
    extra waits onto standalone EventSemaphore NOPs inserted just before the
    offending instruction on the same engine (engines execute in order)."""
    uid = [0]
    for fn in nc.m.functions:
        for bb in fn.blocks:
            il = bb.instructions
            i = 0
            while i < len(il):
                inst = il[i]
                si = inst.sync_info
                waits = list(si.on_wait) if si is not None and si.on_wait else []
                if len(waits) > 1:
                    carriers = []
                    for w in waits[:-1]:
                        uid[0] += 1
                        es = mybir.InstEventSemaphore(
                            name=f"mwsplit_{uid[0]}",
                            engine=inst.engine,
                            ins=[], outs=[],
                            sync_info=mybir.SyncInfo(on_wait=[w], on_update=[]),
                        )
                        carriers.append(es)
                    inst.sync_info = mybir.SyncInfo(
                        on_wait=[waits[-1]], on_update=list(si.on_update or [])
                    )
                    il[i:i] = carriers
                    i += len(carriers)
                i += 1


def _drive(gen, n):
    for _ in range(n):
        try:
            next(gen)
        except StopIteration:
            return


def build_kernel():
    _patch_tile_drain()
    nc = bass.Bass()

    xq = nc.declare_dram_parameter("xq", [D, T], F16, isOutput=False)  # queries^T
    xk = nc.declare_dram_parameter("xk", [D, T], F16, isOutput=False)
    xv = nc.declare_dram_parameter("xv", [D, T], F16, isOutput=False)
    wq = nc.declare_dram_parameter("wq", [D, D], F16, isOutput=False)  # (Wq^T@F) [d,e']
    wk = nc.declare_dram_parameter("wk", [D, D], F16, isOutput=False)
    wv = nc.declare_dram_parameter("wv", [D, D], F16, isOutput=False)  # Wv^T
    wo = nc.declare_dram_parameter("wo", [D, D], F16, isOutput=False)  # Wo^T
    bq = nc.declare_dram_parameter("bq", [D], F32, isOutput=False)     # bq@F
    bk = nc.declare_dram_parameter("bk", [D], F32, isOutput=False)
    bo = nc.declare_dram_parameter("bo", [D], F32, isOutput=False)     # bo + Wo@bv
    tmp = nc.declare_dram_parameter("temp", [H], F32, isOutput=False)
    dinv = nc.declare_dram_parameter("dinv", [2, NF, DH], F16, isOutput=False)
    idn = nc.declare_dram_parameter("idn", [128, 128], F16, isOutput=False)
    out = nc.declare_dram_parameter("out", [T, D], F32, isOutput=True)

    def bcast_ap(param, n):
        return bass.AP(tensor=param, offset=0, ap=[[0, 128], [1, n]])

    with TileContext(nc) as tc:
        import contextlib

        with contextlib.ExitStack() as ctx:
            consts = ctx.enter_context(tc.tile_pool(name="consts", bufs=1))
            qkpool = ctx.enter_context(tc.tile_pool(name="qkpool", bufs=1))
            outfpool = ctx.enter_context(tc.tile_pool(name="outfpool", bufs=1))

            # ---- hot constants (tiny; needed by first evacuations) ----
            bq_sb = consts.tile([128, EC], F32)
            bk_sb = consts.tile([128, EC], F32)
            for bsb, bpar in ((bq_sb, bq), (bk_sb, bk)):
                nc.sync.dma_start(out=bsb, in_=bpar[:].rearrange("(ec p) -> p ec", p=128))
            temp_bc = consts.tile([128, H], F32)
            nc.sync.dma_start(out=temp_bc, in_=bcast_ap(tmp, H))
            tinv = consts.tile([128, H], F32)
            nc.vector.reciprocal(tinv, temp_bc)
            ntinv = consts.tile([128, H], F32)
            nc.vector.tensor_scalar_mul(ntinv, tinv, -1.0)
            ident16 = consts.tile([128, 128], F16)
            Ci_sb = consts.tile([128, DH], F16, name="dinv0")
            Si_sb = consts.tile([128, DH], F16, name="dinv1")

            q16 = qkpool.tile([128, EC, T], F16, tag="q16")   # spectrum Q^T [e', t]
            k16 = qkpool.tile([128, EC, T], F16, tag="k16")
            outf16 = outfpool.tile([128, EC, T], F16)         # Out_f^T [e, t]

            def w2_load(eng, pool, tag, bufs, wpar, dcp, c0, cw, uname):
                """One DMA loading a dc-PAIR of weight rows [128, 2, cw]."""
                w2 = pool.tile([128, 2, cw], F16, tag=tag, bufs=bufs,
                               name=uname)
                eng.dma_start(
                    out=w2,
                    in_=wpar[dcp * 256:(dcp + 1) * 256, c0:c0 + cw]
                    .rearrange("(j p) c -> p j c", p=128))
                return w2

            # ---------- Phase 1: Q/K spectrum projections over full T ----------
            with tc.tile_pool(name="xqk", bufs=1) as xqk, \
                 tc.tile_pool(name="wstg1", bufs=1) as wstg1, \
                 tc.tile_pool(name="psP", bufs=8, space="PSUM") as psP:
                xq16 = xqk.tile([128, DC, T], F16, tag="xq16")
                xk16 = xqk.tile([128, DC, T], F16, tag="xk16")

                def qk_proj(dst16, x16, wpar, bsb, inline_x, after_ecg0):
                    for ecg in range(EC // 4):
                        pss = [psP.tile([128, TH], F32, tag="ps_proj",
                                        name=f"psp_{dst16.name}_{ecg}_{jt}")
                               for jt in range(8)]
                        for dcp in range(DC // 2):
                            if inline_x is not None and ecg == 0:
                                # stream this dc-pair of x just ahead of use
                                for j2 in range(2):
                                    dc = dcp * 2 + j2
                                    for h2 in range(2):
                                        nc.sync.dma_start(
                                            out=x16[:, dc, h2 * TH:(h2 + 1) * TH],
                                            in_=inline_x[dc * 128:(dc + 1) * 128,
                                                         h2 * TH:(h2 + 1) * TH])
                            w2 = w2_load(nc.sync, wstg1, "w16", 8, wpar,
                                         dcp, ecg * 512, 512,
                                         f"w_{dst16.name}_{ecg}_{dcp}")
                            for j2 in range(2):
                                dc = dcp * 2 + j2
                                for j in range(4):
                                    for tn in range(2):
                                        nc.tensor.matmul(
                                            pss[j * 2 + tn],
                                            w2[:, j2, j * 128:(j + 1) * 128],
                                            x16[:, dc, tn * TH:(tn + 1) * TH],
                                            start=(dc == 0), stop=(dc == DC - 1))
                        if ecg == 0 and after_ecg0 is not None:
                            after_ecg0()
                        for j in range(4):
                            ec = ecg * 4 + j
                            for tn in range(2):
                                if j < 2:
                                    nc.scalar.activation(
                                        dst16[:, ec, tn * TH:(tn + 1) * TH],
                                        pss[j * 2 + tn], AF.Identity,
                                        bias=bsb[:, ec:ec + 1])
                                else:
                                    nc.vector.tensor_scalar_add(
                                        dst16[:, ec, tn * TH:(tn + 1) * TH],
                                        pss[j * 2 + tn], bsb[:, ec:ec + 1])

                def _load_rest():
                    # cold constants + K inputs, issued behind Q's first pass
                    nc.sync.dma_start(out=ident16, in_=idn[:])
                    nc.sync.dma_start(out=Ci_sb, in_=dinv[0, :, :])
                    nc.sync.dma_start(out=Si_sb, in_=dinv[1, :, :])
                    for dc in range(DC):
                        nc.sync.dma_start(out=xk16[:, dc, :],
                                          in_=xk[dc * 128:(dc + 1) * 128, :])

                qk_proj(q16, xq16, wq, bq_sb, xq, _load_rest)
                qk_proj(k16, xk16, wk, bk_sb, None, None)

            # ---------- Phases 2-5 ----------
            with tc.tile_pool(name="hpool", bufs=1) as hpool, \
                 tc.tile_pool(name="epool", bufs=1) as epool, \
                 tc.tile_pool(name="small", bufs=8) as small, \
                 tc.tile_pool(name="ypool", bufs=3) as ypool, \
                 tc.tile_pool(name="vpool", bufs=1) as vpool, \
                 tc.tile_pool(name="wstg2", bufs=1) as wstg2:
                v16 = [vpool.tile([128, TH // 128, D], F16, tag=f"v16_{hf}",
                                  name=f"v16_{hf}")
                       for hf in range(2)]

                def v_steps(half):
                    """V projection for one half, token-major; yields ~36x."""
                    t0 = half * TH
                    for ecg in range(EC // 4):
                        psv = [psV.tile([128, 512], F32, tag="ps_vproj",
                                        name=f"psv_{half}_{ecg}_{t}")
                               for t in range(4)]
                        for dcp in range(DC // 2):
                            w2 = w2_load(nc.sync, wstg2, "wstream", 8, wv,
                                         dcp, ecg * 512, 512, f"wv_{half}_{ecg}_{dcp}")
                            for j2 in range(2):
                                dc = dcp * 2 + j2
                                for tck in range(4):
                                    nc.tensor.matmul(
                                        psv[tck],
                                        xv16[:, dc, t0 + tck * 128:t0 + (tck + 1) * 128],
                                        w2[:, j2, :],
                                        start=(dc == 0), stop=(dc == DC - 1))
                            yield
                        for tck in range(4):
                            dst = v16[half][:, tck, ecg * 512:(ecg + 1) * 512]
                            if tck % 2 == 0:
                                nc.scalar.activation(dst, psv[tck], AF.Identity)
                            else:
                                nc.vector.tensor_copy(dst, psv[tck])
                        yield

                def o_steps(half, nbank):
                    """O projection for one half; nbank PSUM banks; yields.
                    half 1 (the tail) issues its DMAs from the scalar queue,
                    which is idle then, to dodge the sync-queue backlog."""
                    t0 = half * TH
                    eng = nc.scalar if half == 1 else nc.sync
                    nsplit = 2 if half == 1 else 1
                    ocw = nbank // 4          # 512-col groups per pass
                    for ocp in range(4 // ocw):
                        psy = [psY[half].tile([128, 512], F32, tag="ps_y",
                                              name=f"psy_{half}_{ocp}_{i}")
                               for i in range(nbank)]
                        c0 = ocp * ocw * 512
                        boc = []
                        for oc in range(ocw):
                            bt = ypool.tile([128, 512], F32, tag="boc", bufs=2,
                                            name=f"boc_{half}_{ocp}_{oc}")
                            eng.dma_start(
                                out=bt,
                                in_=bass.AP(tensor=bo, offset=c0 + oc * 512,
                                            ap=[[0, 128], [1, 512]]))
                            boc.append(bt)
                        for ecp in range(EC // 2):
                            w2s = [w2_load(eng, wstg2, "wstream", 8, wo, ecp,
                                           c0 + oc * 512, 512,
                                           f"wo_{half}_{ocp}_{ecp}_{oc}")
                                   for oc in range(ocw)]
                            for j2 in range(2):
                                ec = ecp * 2 + j2
                                for tck in range(4):
                                    for oc in range(ocw):
                                        nc.tensor.matmul(
                                            psy[tck * ocw + oc],
                                            outf16[:, ec,
                                                   t0 + tck * 128:t0 + (tck + 1) * 128],
                                            w2s[oc][:, j2, :],
                                            start=(ec == 0), stop=(ec == EC - 1))
                            yield
                        for tck in range(4):
                            for oc in range(ocw):
                                cc = c0 + oc * 512
                                yt = ypool.tile([128, 512], F32, tag="yt", bufs=3)
                                nc.vector.tensor_add(yt, psy[tck * ocw + oc],
                                                     boc[oc][:])
                                for j in range(nsplit):
                                    w = 512 // nsplit
                                    eng.dma_start(
                                        out=out[t0 + tck * 128:t0 + (tck + 1) * 128,
                                                cc + j * w:cc + (j + 1) * w],
                                        in_=yt[:, j * w:(j + 1) * w])
                        yield

                def emit_b_head(h, half, fill):
                    """Correlation spectrum -> softmax -> TDA for one head/half.
                    fill: generator driven at stall-prone points."""
                    t0 = half * TH
                    qr = q16[:, 2 * h, t0:t0 + TH]
                    qi = q16[:, 2 * h + 1, t0:t0 + TH]
                    kr = k16[:, 2 * h, t0:t0 + TH]
                    ki = k16[:, 2 * h + 1, t0:t0 + TH]
                    pr = hpool.tile([128, TH], F16, tag="pr", bufs=2)
                    pi = hpool.tile([128, TH], F16, tag="pi", bufs=2)
                    tA = hpool.tile([128, TH], F16, tag="tA", bufs=1)
                    tB = hpool.tile([128, TH], F16, tag="tB", bufs=1)
                    nc.vector.tensor_mul(pr, qr, kr)
                    nc.vector.tensor_mul(tA, qi, ki)
                    nc.vector.tensor_mul(pi, qi, kr)
                    nc.vector.tensor_mul(tB, qr, ki)
                    nc.vector.tensor_add(pr, pr, tA)
                    nc.vector.tensor_sub(pi, pi, tB)

                    # spectrum -> corr (pairs of tck share one PSUM bank; the
                    # pair tile is reused only across heads, after its reads)
                    pscp = []
                    for hp in range(2):
                        pp = psB.tile([128, 2, DH], F32, tag="ps_corr")
                        for j in range(2):
                            tck = hp * 2 + j
                            nc.tensor.matmul(pp[:, j, :],
                                             pr[:, tck * 128:(tck + 1) * 128],
                                             Ci_sb[:], start=True, stop=False)
                            nc.tensor.matmul(pp[:, j, :],
                                             pi[:, tck * 128:(tck + 1) * 128],
                                             Si_sb[:], start=False, stop=True)
                        pscp.append(pp)
                        _drive(fill, 1)
                    _drive(fill, 1)

                    pst = psT.tile([128, 8, 128], F16, tag="ps_et")
                    et16 = epool.tile([128, 2, TH], F16, tag="et16", bufs=2)
                    for tck in range(4):
                        psc = pscp[tck // 2][:, tck % 2, :]
                        mx = small.tile([128, 1], F32, tag="mx")
                        nc.vector.reduce_max(mx, psc, axis=AX.X)
                        nbias = small.tile([128, 1], F32, tag="nbias")
                        nc.vector.tensor_scalar_mul(nbias, mx, ntinv[:, h:h + 1])
                        e16 = epool.tile([128, DH], F16, tag="e16", bufs=2)
                        ssum = small.tile([128, 1], F32, tag="ssum")
                        nc.scalar.activation(e16, psc, AF.Exp,
                                             bias=nbias[:], scale=tinv[:, h:h + 1],
                                             accum_out=ssum[:])
                        rinv = small.tile([128, 1], F32, tag="rinv")
                        nc.vector.reciprocal(rinv, ssum)
                        en16 = epool.tile([128, DH], F16, tag="en16", bufs=2)
                        nc.vector.tensor_scalar_mul(en16, e16, rinv)
                        for sc in range(2):
                            nc.tensor.transpose(pst[:, sc * 4 + tck, :],
                                                en16[:, sc * 128:(sc + 1) * 128],
                                                ident16[:])
                    for sc in range(2):
                        nc.vector.tensor_copy(
                            et16[:, sc, :].rearrange("p (tck c) -> p tck c", c=128),
                            pst[:, sc * 4:(sc + 1) * 4, :])
                    _drive(fill, 1)

                    # TDA: Outf^T[i, t] += Vp[s,i].T @ E^T[s,t] per local batch
                    for b in range(TH // L):
                        pso = psO.tile([128, 2, L], F32, tag="ps_tda")
                        for ic in range(2):
                            for sc in range(2):
                                nc.tensor.matmul(
                                    pso[:, ic, :],
                                    v16[half][:, b * 2 + sc,
                                              h * DH + ic * 128:h * DH + (ic + 1) * 128],
                                    et16[:, sc, b * L:(b + 1) * L],
                                    start=(sc == 0), stop=(sc == 1))
                        for ic in range(2):
                            nc.scalar.activation(
                                outf16[:, 2 * h + ic, t0 + b * L:t0 + (b + 1) * L],
                                pso[:, ic, :], AF.Identity)
                        _drive(fill, 1)

                with tc.tile_pool(name="psB", bufs=2, space="PSUM") as psB, \
                     tc.tile_pool(name="psT", bufs=1, space="PSUM") as psT, \
                     tc.tile_pool(name="psO", bufs=1, space="PSUM") as psO:
                    with tc.tile_pool(name="xvpool", bufs=1) as xvpool, \
                         tc.tile_pool(name="psV", bufs=4, space="PSUM") as psV:
                        xv16 = xvpool.tile([128, DC, T], F16)
                        for dc in range(DC):
                            nc.sync.dma_start(out=xv16[:, dc, :],
                                              in_=xv[dc * 128:(dc + 1) * 128, :])
                        # Phase 2: V(h0) standalone
                        for _ in v_steps(0):
                            pass
                        # Phase 3: B(h0) interleaved with V(h1)
                        vgen = v_steps(1)
                        for h in range(H):
                            emit_b_head(h, 0, vgen)
                        _drive(vgen, 40)

                    # Phase 4: B(h1) interleaved with O(h0)
                    psY = {0: None, 1: None}
                    with tc.tile_pool(name="psY0", bufs=4, space="PSUM") as psY0:
                        psY[0] = psY0
                        ogen = o_steps(0, 4)
                        for h in range(H):
                            emit_b_head(h, 1, ogen)
                        _drive(ogen, 40)

                # Phase 5: O(h1) standalone with 8 banks
                with tc.tile_pool(name="psY1", bufs=8, space="PSUM") as psY1:
                    psY[1] = psY1
                    for _ in o_steps(1, 8):
                        pass

    _split_multiwaits(nc)
    return nc


_NC_CACHE = None


def _get_nc():
    global _NC_CACHE
    if _NC_CACHE is None:
        _NC_CACHE = build_kernel()
    return _NC_CACHE


def _dft_consts():
    m = np.arange(DH, dtype=np.float64)
    f = np.arange(1, NF + 1, dtype=np.float64)   # freqs 1..128 (DC dropped)
    ang_f = 2.0 * np.pi * np.outer(m, f) / DH
    C = np.cos(ang_f)             # [m, NF]
    S = -np.sin(ang_f)
    F = np.concatenate([C, S], axis=1)           # [DH, 2*NF] per-head fold
    n = np.arange(DH, dtype=np.float64)
    w = np.where(f < NF, 2.0, 1.0)[:, None]      # conj-symmetry wts; Nyquist=1
    ang_i = 2.0 * np.pi * np.outer(f, n) / DH
    Ci = w * np.cos(ang_i) / DH   # [NF, n]
    Si = -w * np.sin(ang_i) / DH
    dinv = np.stack([Ci, Si]).astype(np.float16)
    return F, dinv


def _fold_w(Wt, F):
    """Wt [D, H*DH] -> per-head Wt_block @ F, fp16."""
    Wf = np.einsum("dhm,me->dhe", Wt.reshape(D, H, DH).astype(np.float32),
                   F.astype(np.float32))
    return np.ascontiguousarray(Wf.reshape(D, D)).astype(np.float16)


def make_in_maps(inputs):
    F, dinv = _dft_consts()
    idn = np.eye(128, dtype=np.float16)
    Wq = np.asarray(inputs["Wq"], np.float32)
    Wk = np.asarray(inputs["Wk"], np.float32)
    bqF = (np.asarray(inputs["bq"], np.float64).reshape(H, DH) @ F).reshape(D)
    bkF = (np.asarray(inputs["bk"], np.float64).reshape(H, DH) @ F).reshape(D)
    boF = np.asarray(inputs["bo"], np.float64) + \
        np.asarray(inputs["Wo"], np.float64) @ np.asarray(inputs["bv"], np.float64)
    shared = {
        "wq": _fold_w(Wq.T, F),
        "wk": _fold_w(Wk.T, F),
        "wv": np.ascontiguousarray(np.asarray(inputs["Wv"], np.float32).T).astype(np.float16),
        "wo": np.ascontiguousarray(np.asarray(inputs["Wo"], np.float32).T).astype(np.float16),
        "bq": bqF.astype(np.float32),
        "bk": bkF.astype(np.float32),
        "bo": boF.astype(np.float32),
        "temp": np.ascontiguousarray(np.asarray(inputs["temperature"], np.float32).reshape(H)),
        "dinv": dinv,
        "idn": idn,
    }
    in_maps = []
    for c in range(NCORES):
        sl = slice(c * BPC, (c + 1) * BPC)
        m = dict(shared)
        for key, name in (("queries", "xq"), ("keys", "xk"), ("values", "xv")):
            x = np.asarray(inputs[key], np.float32)[sl].reshape(T, D)
            m[name] = np.ascontiguousarray(x.T).astype(np.float16)
        in_maps.append(m)
    return in_maps


def kernel(**inputs):
    nc = _get_nc()
    in_maps = make_in_maps(inputs)
    res = run_bass_kernel_spmd(nc, in_maps, list(range(NCORES)))
    outs = [res.results[i]["out"].reshape(BPC, L, D) for i in range(NCORES)]
    return np.concatenate(outs, axis=0).astype(np.float32, copy=False)


# revision 28
# speedup vs baseline: 1.3293x; 1.0357x over previous
"""AutoCorrelationLayer kernel for 8 TRN2 NeuronCores.

Math (per reference): Q/K/V projections (D=2048, H=8 heads, DH=256),
circular cross-correlation along the head dim per (b,h,l), softmax over the
correlation axis, time-delay aggregation (per-(b,h) 256x256 matmul with V),
output projection.

Key restructurings vs a direct translation:
- The forward DFT acts on the projection OUTPUT features, so it is folded
  into Wq/Wk on the host (WqF = Wq^T @ blockdiag(F)); the device projections
  directly produce the half-spectrum (cos/sin) features at zero extra cost.
  The DC bin is softmax-invariant and dropped; freqs 1..128 retained.
- bv contributes bv @ Wo^T to the output (softmax rows sum to 1), so it is
  folded into bo on the host; V/TDA paths carry no bias.
- All weights and activations are converted to fp16 on the host: no on-chip
  casts, half the HBM traffic. PSUM accumulation is fp32.
- Distribution: pure data-parallel over batch (B=32 -> 4 batches/core).
- Scheduling: V-projection (second half) and O-projection (first half)
  matmuls are interleaved into the per-head correlation/softmax/TDA phases so
  the tensor engine stays continuously busy (avoids p-state down-ramp).
"""

import numpy as np

import concourse.bass as bass
import concourse.mybir as mybir
import concourse.tile as tile_mod
from concourse.tile import TileContext
from concourse.vector_clock import ScopedClock
from concourse.bass_utils import run_bass_kernel_spmd

F32 = mybir.dt.float32
F16 = mybir.dt.float16
AF = mybir.ActivationFunctionType
AX = mybir.AxisListType

B, L, D, H = 32, 256, 2048, 8
DH = D // H          # 256
NCORES = 8
BPC = B // NCORES    # 4 batches per core
T = BPC * L          # 1024 tokens per core
NHALF = 2
TH = T // NHALF      # 512 tokens per half
EC = D // 128        # 16 feature chunks
DC = D // 128        # 16 contraction chunks
NF = 128             # retained spectrum bins (freqs 1..128)


def _patch_tile_drain():
    """This walrus build allows at most ONE semaphore wait per instruction;
    Tile's kernel-tail drain collects one wait per live semaphore on a single
    Drain.  Split the extras onto additional drain instructions."""
    if getattr(tile_mod.TileContext, "_drain_split_patched", False):
        return

    def _drain_and_barrier(self, tick_clock, wait_clock):
        nc = self.nc
        drain_inst = nc.sync.drain()
        wait_clock.add_sem_waits(
            drain_inst.ins, ScopedClock({None: tick_clock.global_clock})
        )
        si = drain_inst.ins.sync_info
        waits = list(si.on_wait) if si is not None and si.on_wait else []
        if len(waits) > 1:
            drain_inst.ins.sync_info = mybir.SyncInfo(
                on_wait=[waits[0]], on_update=list(si.on_update or [])
            )
            for w in waits[1:]:
                extra = nc.sync.drain()
                extra.ins.sync_info = mybir.SyncInfo(on_wait=[w], on_update=[])
        nc.all_engine_barrier()
        popped = nc._tile_sem_poison_stack.pop()
        assert popped is self._sem_poison
        nc.clear_and_free_semaphores(list(self.sems.allocated().values()))
        nc.all_engine_barrier()

    tile_mod.TileContext._drain_and_barrier = _drain_and_barrier
    tile_mod.TileContext._drain_split_patched = True


def _split_multiwaits(nc):
    """Walrus in this build rejects >1 semaphore wait per instruction.  Hoist
    extra waits onto standalone EventSemaphore NOPs inserted just before the
    offending instruction on the same engine (engines execute in order)."""
    uid = [0]
    for fn in nc.m.functions:
        for bb in fn.blocks:
            il = bb.instructions
            i = 0
            while i < len(il):
                inst = il[i]
                si = inst.sync_info
                waits = list(si.on_wait) if si is not None and si.on_wait else []
                if len(waits) > 1:
                    carriers = []
                    for w in waits[:-1]:
                        uid[0] += 1
                        es = mybir.InstEventSemaphore(
                            name=f"mwsplit_{uid[0]}",
                            engine=inst.engine,
                            ins=[], outs=[],
                            sync_info=mybir.SyncInfo(on_wait=[w], on_update=[]),
                        )
                        carriers.append(es)
                    inst.sync_info = mybir.SyncInfo(
                        on_wait=[waits[-1]], on_update=list(si.on_update or [])
                    )
                    il[i:i] = carriers
                    i += len(carriers)
                i += 1


def _drive(gen, n):
    for _ in range(n):
        try:
            next(gen)
        except StopIteration:
            return


def build_kernel():
    _patch_tile_drain()
    nc = bass.Bass()

    xq = nc.declare_dram_parameter("xq", [D, T], F16, isOutput=False)  # queries^T
    xk = nc.declare_dram_parameter("xk", [D, T], F16, isOutput=False)
    xv = nc.declare_dram_parameter("xv", [D, T], F16, isOutput=False)
    wq = nc.declare_dram_parameter("wq", [D, D], F16, isOutput=False)  # (Wq^T@F) [d,e']
    wk = nc.declare_dram_parameter("wk", [D, D], F16, isOutput=False)
    wv = nc.declare_dram_parameter("wv", [D, D], F16, isOutput=False)  # Wv^T
    wo = nc.declare_dram_parameter("wo", [D, D], F16, isOutput=False)  # Wo^T
    bq = nc.declare_dram_parameter("bq", [D], F32, isOutput=False)     # bq@F
    bk = nc.declare_dram_parameter("bk", [D], F32, isOutput=False)
    bo = nc.declare_dram_parameter("bo", [D], F32, isOutput=False)     # bo + Wo@bv
    tmp = nc.declare_dram_parameter("temp", [H], F32, isOutput=False)
    dinv = nc.declare_dram_parameter("dinv", [2, NF, DH], F16, isOutput=False)
    idn = nc.declare_dram_parameter("idn", [128, 128], F16, isOutput=False)
    out = nc.declare_dram_parameter("out", [D, T], F32, isOutput=True)

    def bcast_ap(param, n):
        return bass.AP(tensor=param, offset=0, ap=[[0, 128], [1, n]])

    with TileContext(nc) as tc:
        import contextlib

        with contextlib.ExitStack() as ctx:
            consts = ctx.enter_context(tc.tile_pool(name="consts", bufs=1))
            qkpool = ctx.enter_context(tc.tile_pool(name="qkpool", bufs=1))
            outfpool = ctx.enter_context(tc.tile_pool(name="outfpool", bufs=1))

            # ---- hot constants (tiny; needed by first evacuations) ----
            bq_sb = consts.tile([128, EC], F32)
            bk_sb = consts.tile([128, EC], F32)
            bo_sb = consts.tile([128, EC], F32)
            for bsb, bpar in ((bq_sb, bq), (bk_sb, bk), (bo_sb, bo)):
                nc.sync.dma_start(out=bsb, in_=bpar[:].rearrange("(ec p) -> p ec", p=128))
            temp_bc = consts.tile([128, H], F32)
            nc.sync.dma_start(out=temp_bc, in_=bcast_ap(tmp, H))
            tinv = consts.tile([128, H], F32)
            nc.vector.reciprocal(tinv, temp_bc)
            ntinv = consts.tile([128, H], F32)
            nc.vector.tensor_scalar_mul(ntinv, tinv, -1.0)
            ident16 = consts.tile([128, 128], F16)
            Ci_sb = consts.tile([128, DH], F16, name="dinv0")
            Si_sb = consts.tile([128, DH], F16, name="dinv1")

            q16 = qkpool.tile([128, EC, T], F16, tag="q16")   # spectrum Q^T [e', t]
            k16 = qkpool.tile([128, EC, T], F16, tag="k16")
            outf16 = outfpool.tile([128, EC, T], F16)         # Out_f^T [e, t]

            def w2_load(eng, pool, tag, bufs, wpar, dcp, c0, cw, uname):
                """One DMA loading a dc-PAIR of weight rows [128, 2, cw]."""
                w2 = pool.tile([128, 2, cw], F16, tag=tag, bufs=bufs,
                               name=uname)
                eng.dma_start(
                    out=w2,
                    in_=wpar[dcp * 256:(dcp + 1) * 256, c0:c0 + cw]
                    .rearrange("(j p) c -> p j c", p=128))
                return w2

            # ---------- Phase 1: Q/K spectrum projections over full T ----------
            with tc.tile_pool(name="xqk", bufs=1) as xqk, \
                 tc.tile_pool(name="wstg1", bufs=1) as wstg1, \
                 tc.tile_pool(name="psP", bufs=8, space="PSUM") as psP:
                xq16 = xqk.tile([128, DC, T], F16, tag="xq16")
                xk16 = xqk.tile([128, DC, T], F16, tag="xk16")

                def _cold_loads():
                    # cold constants + K inputs, dribbled out one per dc-pair
                    # behind Q's later passes so they never block weight issue
                    yield nc.sync.dma_start(out=ident16, in_=idn[:])
                    yield nc.sync.dma_start(out=Ci_sb, in_=dinv[0, :, :])
                    yield nc.sync.dma_start(out=Si_sb, in_=dinv[1, :, :])
                    for dc in range(DC):
                        yield nc.sync.dma_start(out=xk16[:, dc, :],
                                                in_=xk[dc * 128:(dc + 1) * 128, :])

                def qk_proj(dst16, x16, wpar, bsb, inline_x, dribble):
                    for ecg in range(EC // 4):
                        pss = [psP.tile([128, TH], F32, tag="ps_proj",
                                        name=f"psp_{dst16.name}_{ecg}_{jt}")
                               for jt in range(8)]
                        for dcp in range(DC // 2):
                            if inline_x is not None and ecg == 0:
                                # stream this dc-pair of x just ahead of use
                                for j2 in range(2):
                                    dc = dcp * 2 + j2
                                    for h2 in range(2):
                                        nc.sync.dma_start(
                                            out=x16[:, dc, h2 * TH:(h2 + 1) * TH],
                                            in_=inline_x[dc * 128:(dc + 1) * 128,
                                                         h2 * TH:(h2 + 1) * TH])
                            w2 = w2_load(nc.sync, wstg1, "w16", 8, wpar,
                                         dcp, ecg * 512, 512,
                                         f"w_{dst16.name}_{ecg}_{dcp}")
                            if ecg > 0 and dribble is not None:
                                _drive(dribble, 1)
                            for j2 in range(2):
                                dc = dcp * 2 + j2
                                for j in range(4):
                                    for tn in range(2):
                                        nc.tensor.matmul(
                                            pss[j * 2 + tn],
                                            w2[:, j2, j * 128:(j + 1) * 128],
                                            x16[:, dc, tn * TH:(tn + 1) * TH],
                                            start=(dc == 0), stop=(dc == DC - 1))
                        for j in range(4):
                            ec = ecg * 4 + j
                            for tn in range(2):
                                if j < 2:
                                    nc.scalar.activation(
                                        dst16[:, ec, tn * TH:(tn + 1) * TH],
                                        pss[j * 2 + tn], AF.Identity,
                                        bias=bsb[:, ec:ec + 1])
                                else:
                                    nc.vector.tensor_scalar_add(
                                        dst16[:, ec, tn * TH:(tn + 1) * TH],
                                        pss[j * 2 + tn], bsb[:, ec:ec + 1])

                cold = _cold_loads()
                qk_proj(q16, xq16, wq, bq_sb, xq, cold)
                _drive(cold, 25)
                qk_proj(k16, xk16, wk, bk_sb, None, None)

            # ---------- Phases 2-5 ----------
            with tc.tile_pool(name="hpool", bufs=1) as hpool, \
                 tc.tile_pool(name="epool", bufs=1) as epool, \
                 tc.tile_pool(name="small", bufs=8) as small, \
                 tc.tile_pool(name="ypool", bufs=3) as ypool, \
                 tc.tile_pool(name="vpool", bufs=1) as vpool, \
                 tc.tile_pool(name="wstg2", bufs=1) as wstg2:
                v16 = [vpool.tile([128, TH // 128, D], F16, tag=f"v16_{hf}",
                                  name=f"v16_{hf}")
                       for hf in range(2)]

                def v_steps(half):
                    """V projection for one half, token-major; yields ~36x."""
                    t0 = half * TH
                    for ecg in range(EC // 4):
                        psv = [psV.tile([128, 512], F32, tag="ps_vproj",
                                        name=f"psv_{half}_{ecg}_{t}")
                               for t in range(4)]
                        for dcp in range(DC // 2):
                            w2 = w2_load(nc.sync, wstg2, "wstream", 8, wv,
                                         dcp, ecg * 512, 512, f"wv_{half}_{ecg}_{dcp}")
                            for j2 in range(2):
                                dc = dcp * 2 + j2
                                for tck in range(4):
                                    nc.tensor.matmul(
                                        psv[tck],
                                        xv16[:, dc, t0 + tck * 128:t0 + (tck + 1) * 128],
                                        w2[:, j2, :],
                                        start=(dc == 0), stop=(dc == DC - 1))
                            yield
                        for tck in range(4):
                            dst = v16[half][:, tck, ecg * 512:(ecg + 1) * 512]
                            if tck % 2 == 0:
                                nc.scalar.activation(dst, psv[tck], AF.Identity)
                            else:
                                nc.vector.tensor_copy(dst, psv[tck])
                        yield

                def o_steps(half, nbank):
                    """O projection for one half, OUTPUT-FEATURE-major:
                    psy[o, t] = sum_e wo[e, o] * outf16[e, t].  Bias is then
                    per-partition (scalar activation) and out rows are
                    contiguous in the [D, T] out tensor.  half 1 (the tail)
                    issues DMAs from the scalar queue, which is idle then."""
                    t0 = half * TH
                    eng = nc.scalar if half == 1 else nc.sync
                    for ocg in range(16 // nbank):
                        psy = [psY[half].tile([128, TH], F32, tag="ps_y",
                                              name=f"psy_{half}_{ocg}_{i}")
                               for i in range(nbank)]
                        c0 = ocg * nbank * 128
                        for ecp in range(EC // 2):
                            w2s = [w2_load(nc.sync if (half == 1 and ocg == 0)
                                           else eng,
                                           wstg2, "wstream", 8, wo, ecp,
                                           c0 + i * 512, 512,
                                           f"wo_{half}_{ocg}_{ecp}_{i}")
                                   for i in range(nbank // 4)]
                            for j2 in range(2):
                                ec = ecp * 2 + j2
                                for ob in range(nbank):
                                    nc.tensor.matmul(
                                        psy[ob],
                                        w2s[ob // 4][:, j2,
                                                     (ob % 4) * 128:(ob % 4 + 1) * 128],
                                        outf16[:, ec, t0:t0 + TH],
                                        start=(ec == 0), stop=(ec == EC - 1))
                            yield
                        for ob in range(nbank):
                            oc = ocg * nbank + ob
                            yt = ypool.tile([128, TH], F32, tag="yt", bufs=3)
                            nc.scalar.activation(yt, psy[ob], AF.Identity,
                                                 bias=bo_sb[:, oc:oc + 1])
                            for j in range(2):
                                qeng = nc.sync if (half == 0 or j == 0) else nc.scalar
                                qeng.dma_start(
                                    out=out[oc * 128:(oc + 1) * 128,
                                            t0 + j * 256:t0 + (j + 1) * 256],
                                    in_=yt[:, j * 256:(j + 1) * 256])
                        yield

                def cmul(h, half):
                    """Cross-spectrum P = Qf * conj(Kf) for one head/half."""
                    t0 = half * TH
                    qr = q16[:, 2 * h, t0:t0 + TH]
                    qi = q16[:, 2 * h + 1, t0:t0 + TH]
                    kr = k16[:, 2 * h, t0:t0 + TH]
                    ki = k16[:, 2 * h + 1, t0:t0 + TH]
                    pr = hpool.tile([128, TH], F16, tag="pr", bufs=2,
                                    name=f"pr_{half}_{h}")
                    pi = hpool.tile([128, TH], F16, tag="pi", bufs=2,
                                    name=f"pi_{half}_{h}")
                    tA = hpool.tile([128, TH], F16, tag="tA", bufs=1,
                                    name=f"tA_{half}_{h}")
                    tB = hpool.tile([128, TH], F16, tag="tB", bufs=1,
                                    name=f"tB_{half}_{h}")
                    nc.vector.tensor_mul(pr, qr, kr)
                    nc.vector.tensor_mul(tA, qi, ki)
                    nc.vector.tensor_mul(pi, qi, kr)
                    nc.vector.tensor_mul(tB, qr, ki)
                    nc.vector.tensor_add(pr, pr, tA)
                    nc.vector.tensor_sub(pi, pi, tB)
                    return pr, pi

                def emit_b_head(h, half, fill, pp=None):
                    """Correlation spectrum -> softmax -> TDA for one head/half.
                    fill: generator driven at stall-prone points."""
                    t0 = half * TH
                    pr, pi = pp if pp is not None else cmul(h, half)

                    # spectrum -> corr (pairs of tck share one PSUM bank; the
                    # pair tile is reused only across heads, after its reads)
                    pscp = []
                    for hp in range(2):
                        pcb = psB.tile([128, 2, DH], F32, tag="ps_corr",
                                       name=f"psc_{half}_{h}_{hp}")
                        for j in range(2):
                            tck = hp * 2 + j
                            nc.tensor.matmul(pcb[:, j, :],
                                             pr[:, tck * 128:(tck + 1) * 128],
                                             Ci_sb[:], start=True, stop=False)
                            nc.tensor.matmul(pcb[:, j, :],
                                             pi[:, tck * 128:(tck + 1) * 128],
                                             Si_sb[:], start=False, stop=True)
                        pscp.append(pcb)
                        _drive(fill, 1)

                    pst = psT.tile([128, 8, 128], F16, tag="ps_et")
                    et16 = epool.tile([128, 2, TH], F16, tag="et16", bufs=2)
                    for tck in range(4):
                        psc = pscp[tck // 2][:, tck % 2, :]
                        mx = small.tile([128, 1], F32, tag="mx")
                        nc.vector.reduce_max(mx, psc, axis=AX.X)
                        nbias = small.tile([128, 1], F32, tag="nbias")
                        nc.vector.tensor_scalar_mul(nbias, mx, ntinv[:, h:h + 1])
                        e16 = epool.tile([128, DH], F16, tag="e16", bufs=2)
                        ssum = small.tile([128, 1], F32, tag="ssum")
                        nc.scalar.activation(e16, psc, AF.Exp,
                                             bias=nbias[:], scale=tinv[:, h:h + 1],
                                             accum_out=ssum[:])
                        rinv = small.tile([128, 1], F32, tag="rinv")
                        nc.vector.reciprocal(rinv, ssum)
                        en16 = epool.tile([128, DH], F16, tag="en16", bufs=2)
                        nc.vector.tensor_scalar_mul(en16, e16, rinv)
                        for sc in range(2):
                            nc.tensor.transpose(pst[:, sc * 4 + tck, :],
                                                en16[:, sc * 128:(sc + 1) * 128],
                                                ident16[:])
                    for sc in range(2):
                        nc.vector.tensor_copy(
                            et16[:, sc, :].rearrange("p (tck c) -> p tck c", c=128),
                            pst[:, sc * 4:(sc + 1) * 4, :])
                    _drive(fill, 1)

                    # TDA: Outf^T[i, t] += Vp[s,i].T @ E^T[s,t] per local batch
                    for b in range(TH // L):
                        pso = psO.tile([128, 2, L], F32, tag="ps_tda")
                        for ic in range(2):
                            for sc in range(2):
                                nc.tensor.matmul(
                                    pso[:, ic, :],
                                    v16[half][:, b * 2 + sc,
                                              h * DH + ic * 128:h * DH + (ic + 1) * 128],
                                    et16[:, sc, b * L:(b + 1) * L],
                                    start=(sc == 0), stop=(sc == 1))
                        for ic in range(2):
                            nc.scalar.activation(
                                outf16[:, 2 * h + ic, t0 + b * L:t0 + (b + 1) * L],
                                pso[:, ic, :], AF.Identity)
                        _drive(fill, 1)

                with tc.tile_pool(name="psB", bufs=2, space="PSUM") as psB, \
                     tc.tile_pool(name="psT", bufs=1, space="PSUM") as psT, \
                     tc.tile_pool(name="psO", bufs=1, space="PSUM") as psO:
                    with tc.tile_pool(name="xvpool", bufs=1) as xvpool, \
                         tc.tile_pool(name="psV", bufs=4, space="PSUM") as psV:
                        xv16 = xvpool.tile([128, DC, T], F16)
                        for dc in range(DC):
                            nc.sync.dma_start(out=xv16[:, dc, :],
                                              in_=xv[dc * 128:(dc + 1) * 128, :])
                        # Phase 2: V(h0) standalone; pre-emit first cross-
                        # spectra so B(h0) starts with zero vector latency
                        pre = {(0, 0): cmul(0, 0), (1, 0): cmul(1, 0)}
                        for _ in v_steps(0):
                            pass
                        # Phase 3: B(h0) interleaved with V(h1)
                        vgen = v_steps(1)
                        for h in range(H):
                            emit_b_head(h, 0, vgen, pre.pop((h, 0), None))
                        pre[(0, 1)] = cmul(0, 1)
                        pre[(1, 1)] = cmul(1, 1)
                        _drive(vgen, 40)

                    # Phase 4: B(h1) interleaved with O(h0)
                    psY = {0: None, 1: None}
                    with tc.tile_pool(name="psY0", bufs=4, space="PSUM") as psY0:
                        psY[0] = psY0
                        ogen = o_steps(0, 4)
                        for h in range(H):
                            emit_b_head(h, 1, ogen, pre.pop((h, 1), None))
                        _drive(ogen, 40)

                # Phase 5: O(h1) standalone with 8 banks
                with tc.tile_pool(name="psY1", bufs=8, space="PSUM") as psY1:
                    psY[1] = psY1
                    for _ in o_steps(1, 8):
                        pass

    _split_multiwaits(nc)
    return nc


_NC_CACHE = None


def _get_nc():
    global _NC_CACHE
    if _NC_CACHE is None:
        _NC_CACHE = build_kernel()
    return _NC_CACHE


def _dft_consts():
    m = np.arange(DH, dtype=np.float64)
    f = np.arange(1, NF + 1, dtype=np.float64)   # freqs 1..128 (DC dropped)
    ang_f = 2.0 * np.pi * np.outer(m, f) / DH
    C = np.cos(ang_f)             # [m, NF]
    S = -np.sin(ang_f)
    F = np.concatenate([C, S], axis=1)           # [DH, 2*NF] per-head fold
    n = np.arange(DH, dtype=np.float64)
    w = np.where(f < NF, 2.0, 1.0)[:, None]      # conj-symmetry wts; Nyquist=1
    ang_i = 2.0 * np.pi * np.outer(f, n) / DH
    Ci = w * np.cos(ang_i) / DH   # [NF, n]
    Si = -w * np.sin(ang_i) / DH
    dinv = np.stack([Ci, Si]).astype(np.float16)
    return F, dinv


def _fold_w(Wt, F):
    """Wt [D, H*DH] -> per-head Wt_block @ F, fp16."""
    Wf = np.einsum("dhm,me->dhe", Wt.reshape(D, H, DH).astype(np.float32),
                   F.astype(np.float32))
    return np.ascontiguousarray(Wf.reshape(D, D)).astype(np.float16)


def make_in_maps(inputs):
    F, dinv = _dft_consts()
    idn = np.eye(128, dtype=np.float16)
    Wq = np.asarray(inputs["Wq"], np.float32)
    Wk = np.asarray(inputs["Wk"], np.float32)
    bqF = (np.asarray(inputs["bq"], np.float64).reshape(H, DH) @ F).reshape(D)
    bkF = (np.asarray(inputs["bk"], np.float64).reshape(H, DH) @ F).reshape(D)
    boF = np.asarray(inputs["bo"], np.float64) + \
        np.asarray(inputs["Wo"], np.float64) @ np.asarray(inputs["bv"], np.float64)
    shared = {
        "wq": _fold_w(Wq.T, F),
        "wk": _fold_w(Wk.T, F),
        "wv": np.ascontiguousarray(np.asarray(inputs["Wv"], np.float32).T).astype(np.float16),
        "wo": np.ascontiguousarray(np.asarray(inputs["Wo"], np.float32).T).astype(np.float16),
        "bq": bqF.astype(np.float32),
        "bk": bkF.astype(np.float32),
        "bo": boF.astype(np.float32),
        "temp": np.ascontiguousarray(np.asarray(inputs["temperature"], np.float32).reshape(H)),
        "dinv": dinv,
        "idn": idn,
    }
    in_maps = []
    for c in range(NCORES):
        sl = slice(c * BPC, (c + 1) * BPC)
        m = dict(shared)
        for key, name in (("queries", "xq"), ("keys", "xk"), ("values", "xv")):
            x = np.asarray(inputs[key], np.float32)[sl].reshape(T, D)
            m[name] = np.ascontiguousarray(x.T).astype(np.float16)
        in_maps.append(m)
    return in_maps


def kernel(**inputs):
    nc = _get_nc()
    in_maps = make_in_maps(inputs)
    res = run_bass_kernel_spmd(nc, in_maps, list(range(NCORES)))
    # device output is [D, T] (feature-major); transpose back on host
    outs = [np.ascontiguousarray(res.results[i]["out"].T).reshape(BPC, L, D)
            for i in range(NCORES)]
    return np.concatenate(outs, axis=0).astype(np.float32, copy=False)
